# revision 1
# baseline (speedup 1.0000x reference)
"""Causal self-attention (weight-modulated) Trainium2 kernel, 8-core SPMD.

Reference semantics (B=2, T=2048, C=512, 8 heads, hd=64):
    v0  = x @ Wv.T + bv
    v   = v0 * w[:, :, None]            # w = weight[:, :, 0]
    att = softmax(mask((v0h @ v0h^T) * w[key] / sqrt(hd)))
    y   = att @ vh
    out = y @ Wp.T + bp

Sharding: core = (b, hp) with b = batch, hp = head pair (v0 dims
[128hp, 128hp+128)).  Unlike a query-sharded dense kernel, every core
exploits causality uniformly: for query chunk qj (512 rows) only key
blocks ki <= 4qj+3 are touched, and diagonal blocks are trimmed to
their triangular span.  Per core:
    vT   = (Wv_hp @ x^T) (+bv)                  [128 hd2, 2048 t]
    va   = vT^T * w[key]  (+ ones cols)         [2048 k, 130]
    S    = vT^T vT  (QK, K=64 row-group pairs)  [128 k, 2x512 q]
    e    = exp(S * w[k]/8)   (tri-masked on diagonal blocks)
    y,d += va^T e            (ones col -> denominator in row 64)
    y_n  = y * recip(d)      (DVE recip + K=1 broadcast matmul + TT)
    outP = Wp_hp^T @ y_n                        [512 c, 2048 q] bf16
Host: out[b] = sum_hp outP^T + bp  (partial-sum reduce off-device).
"""

import ml_dtypes
import numpy as np

B, T, C = 2, 2048, 512
NH, HD = 8, 64
P = 128
QB = 512                 # query chunk
NQ = 4                   # query chunks
NKB = 16                 # key blocks of 128

_cache = {}


def _split_multi_waits(nc, mybir):
    """Walrus in this container encodes at most ONE sync wait (and one
    update) per instruction; Tile's sem assignment emits several. Hoist
    excess waits onto single-wait NOPs placed just before the
    instruction on the same engine, and excess updates of non-DMA
    instructions onto NOPs just after."""
    dma_ops = {"DMACopy", "DMATranspose", "TensorCopy"}
    for f in nc.m.functions:
        for bb in f.blocks:
            new = []
            changed = False
            for inst in bb.instructions:
                si = inst.sync_info
                waits = list(si.on_wait or []) if si is not None else []
                ups = list(si.on_update or []) if si is not None else []
                is_dma = inst.concise_opcode() in dma_ops if hasattr(
                    inst, "concise_opcode") else False
                post = []
                if si is not None and len(waits) > 1:
                    for w in waits[:-1]:
                        nop = mybir.InstNoOp(
                            name=nc.get_next_instruction_name(),
                            sync_info=mybir.SyncInfo(on_wait=[w], on_update=[]),
                            bass_nofuse=True,
                            engine=inst.engine,
                        )
                        nc.register_instruction(nop, overwrite=True)
                        new.append(nop)
                    waits = waits[-1:]
                    inst.sync_info = mybir.SyncInfo(on_wait=waits, on_update=ups)
                    changed = True
                if si is not None and len(ups) > 1 and not is_dma:
                    for u in ups[1:]:
                        nop = mybir.InstNoOp(
                            name=nc.get_next_instruction_name(),
                            sync_info=mybir.SyncInfo(on_wait=[], on_update=[u]),
                            bass_nofuse=True,
                            engine=inst.engine,
                        )
                        nc.register_instruction(nop, overwrite=True)
                        post.append(nop)
                    inst.sync_info = mybir.SyncInfo(
                        on_wait=waits, on_update=ups[:1])
                    changed = True
                new.append(inst)
                new.extend(post)
            if changed:
                bb.instructions = new


def _tri01():
    # U[s, j] = 1 if j >= s else 0 : causal mask for a diagonal
    # [128 key, span query] block whose query span starts at key row 0.
    # Stored twice side by side so a [p, 2, span] strided AP can mask a
    # head pair's two score halves in one op.
    s = np.arange(P)[:, None]
    j = np.arange(QB)[None, :]
    u = (j >= s).astype(ml_dtypes.bfloat16)
    return np.concatenate([u, u], axis=1)


def _build_nc(with_bias):
    import concourse.bass as bass
    import concourse.mybir as mybir
    from concourse.tile import TileContext

    f32 = mybir.dt.float32
    f32r = mybir.dt.float32r
    bf16 = mybir.dt.bfloat16
    AF = mybir.ActivationFunctionType
    ALU = mybir.AluOpType
    i32 = mybir.dt.int32

    nc = bass.Bass()

    # packed inputs: fewer DMA instructions (SP issue is ~600ns each)
    xTp = nc.dram_tensor("xTp", [P, 4 * T], bf16, kind="ExternalInput")
    wvp = nc.dram_tensor("wvp", [P, C], bf16, kind="ExternalInput")
    wpT = nc.dram_tensor("wpT", [P, C], bf16, kind="ExternalInput")
    smf = nc.dram_tensor("smf", [P, 1 + 2 * NKB], f32, kind="ExternalInput")
    outT = nc.dram_tensor("outT", [C, T], bf16, kind="ExternalOutput")

    cb = np.concatenate(
        [np.eye(P).astype(ml_dtypes.bfloat16), _tri01()], axis=1)
    cb_d = nc.inline_tensor(cb, name="cbf16")
    magic_d = nc.inline_tensor(
        np.full((1, QB), 0x7EF311C3, np.int32), name="rmagic")
    pm = np.concatenate([np.full((1, HD), -1.0), np.full((1, HD), 1.0)],
                        axis=1).astype(ml_dtypes.bfloat16)
    pm64_d = nc.inline_tensor(pm, name="pm64")

    with TileContext(nc) as tc:
        with (
            tc.tile_pool(name="persist", bufs=1) as pp,
            tc.tile_pool(name="stream", bufs=3) as sp,
            tc.tile_pool(name="psum", bufs=2, space="PSUM") as qq,
        ):
            # ---- persistent SBUF ----
            xT_sb = pp.tile([P, 4 * T], bf16, tag="xTp")
            vT_sb = pp.tile([P, T], bf16, tag="vT")
            wv_sb = pp.tile([P, C], bf16, tag="wvp")
            wpT_sb = pp.tile([P, C], bf16, tag="wp")
            va_sb = [pp.tile([P, 2 * HD + 2], bf16, tag=f"va{i}",
                             name=f"va{i}") for i in range(NKB)]
            smf_sb = pp.tile([P, 1 + 2 * NKB], f32, tag="smf")
            cb_sb = pp.tile([P, P + 2 * QB], bf16, tag="cbf16")
            magic_sb = pp.tile([1, QB], i32, tag="rmagic")
            pm64_sb = pp.tile([1, 2 * HD], bf16, tag="pm64")
            bvc_sb = smf_sb[:, 0:1]
            w8_sb = smf_sb[:, 1:1 + NKB]
            wvw_sb = smf_sb[:, 1 + NKB:1 + 2 * NKB]
            idn_sb = cb_sb[:, 0:P]
            U_sb = cb_sb[:, P:P + 2 * QB]

            # ---- prologue DMAs (order = need order) ----
            xT3d = xTp.rearrange("p (k t) -> p k t", t=T)
            xT3s = xT_sb[:].rearrange("p (k t) -> p k t", t=T)
            nc.sync.dma_start(out=xT3s[:, 0, 0:QB], in_=xT3d[:, 0, 0:QB])
            nc.sync.dma_start(out=wv_sb[:], in_=wvp[:])
            for k in range(1, 4):
                nc.sync.dma_start(out=xT3s[:, k, 0:QB],
                                  in_=xT3d[:, k, 0:QB])
            nc.sync.dma_start(out=cb_sb[:], in_=cb_d[:])
            nc.sync.dma_start(out=smf_sb[:], in_=smf[:])
            nc.sync.dma_start(out=magic_sb[:], in_=magic_d[:])
            nc.sync.dma_start(out=pm64_sb[:], in_=pm64_d[:])
            for c in range(1, 4):
                nc.sync.dma_start(out=xT3s[:, :, c * QB:(c + 1) * QB],
                                  in_=xT3d[:, :, c * QB:(c + 1) * QB])
            nc.sync.dma_start(out=wpT_sb[:], in_=wpT[:])
            # ones columns of va (cols 64 and 129), set once
            for i in range(NKB):
                nc.gpsimd.memset(va_sb[i][:, HD:HD + 1], 1.0)
                nc.gpsimd.memset(va_sb[i][:, 2 * HD + 1:2 * HD + 2], 1.0)

            ebuf = {}
            yps_t = {}
            dps_t = {}
            ysb_t = {}

            def emit_A(qj):
                # vT chunk qj + va blocks 4qj..4qj+3
                vps = qq.tile([P, QB], f32, tag="mm", name=f"vps{qj}")
                for k in range(4):
                    nc.tensor.matmul(
                        vps[:], wv_sb[:, k * P:(k + 1) * P],
                        xT_sb[:, k * T + qj * QB:k * T + (qj + 1) * QB],
                        start=(k == 0), stop=(k == 3))
                if with_bias:
                    nc.vector.tensor_scalar_add(
                        vT_sb[:, qj * QB:(qj + 1) * QB], vps[:],
                        bvc_sb)
                else:
                    nc.vector.tensor_copy(
                        vT_sb[:, qj * QB:(qj + 1) * QB], vps[:])
                for g in range(4):
                    kb = 4 * qj + g
                    tps = qq.tile([P, P], bf16, tag="mm", name=f"tr{kb}")
                    nc.tensor.transpose(
                        tps[:], vT_sb[:, kb * P:(kb + 1) * P], idn_sb)
                    # one strided op writes both heads' 64-col va windows
                    va3 = va_sb[kb][:, 0:2 * HD + 2].rearrange(
                        "p (u d) -> p u d", d=HD + 1)
                    nc.vector.tensor_scalar_mul(
                        va3[:, :, 0:HD],
                        tps[:].rearrange("p (u d) -> p u d", d=HD),
                        wvw_sb[:, kb:kb + 1])

            def emit_QKexp(qj, ki):
                diag = ki >= 4 * qj
                so = P * (ki - 4 * qj) if diag else 0
                spair = qq.tile([P, 2 * QB], f32, tag="S",
                                name=f"S{qj}_{ki}")
                for u in (0, 1):
                    nc.tensor.matmul(
                        spair[:, u * QB + so:(u + 1) * QB],
                        vT_sb[HD * u:HD * (u + 1), ki * P:(ki + 1) * P],
                        vT_sb[HD * u:HD * (u + 1),
                              qj * QB + so:(qj + 1) * QB],
                        start=True, stop=True)
                e = sp.tile([P, 2 * QB], bf16, tag="e", name=f"e{qj}_{ki}",
                            bufs=6)
                if not diag:
                    nc.scalar.activation(
                        e[:], spair[:], AF.Exp, scale=w8_sb[:, ki:ki + 1])
                else:
                    span = QB - so
                    e3 = e[:].rearrange("p (u q) -> p u q", q=QB)
                    s3 = spair[:].rearrange("p (u q) -> p u q", q=QB)
                    u3 = U_sb.rearrange("p (u q) -> p u q", q=QB)
                    nc.scalar.activation(
                        e3[:, :, so:QB], s3[:, :, so:QB],
                        AF.Exp, scale=w8_sb[:, ki:ki + 1])
                    # per-head mask: u0 on DVE (fast, unblocks AV first),
                    # u1 on Pool (otherwise idle)
                    for u, eng in ((0, nc.vector), (1, nc.gpsimd)):
                        eng.tensor_mul(
                            e[:, u * QB + so:(u + 1) * QB],
                            e[:, u * QB + so:(u + 1) * QB],
                            U_sb[:, 0:span])
                ebuf[(qj, ki)] = (e, so)

            def emit_AV(qj, ki):
                e, so = ebuf.pop((qj, ki))
                if ki == 0:
                    yps_t[qj] = [
                        qq.tile([HD + 1, QB], f32, tag="yps",
                                name=f"yps{qj}_{u}") for u in (0, 1)]
                for u in (0, 1):
                    nc.tensor.matmul(
                        yps_t[qj][u][0:HD + 1, so:QB],
                        va_sb[ki][:, u * (HD + 1):(u + 1) * (HD + 1)],
                        e[:, u * QB + so:(u + 1) * QB],
                        start=(ki == 0), stop=(ki == 4 * qj + 3))

            def emit_recips_dps(qj):
                # Both heads' broadcast reciprocals share one [128, 512]
                # psum tile: rows 0:64 = 1/d0, rows 64:128 = 1/d1.
                # 1/d via bit-trick seed + one Newton pass on DVE (max rel
                # err ~0.26%); the sign of -r1 folds into the -1 weights
                # of the K=1 broadcast matmul.  For the last qj the DVE
                # chain is the exposed tail, so use the then-idle ACT
                # engine instead: 1/d = exp(-ln d).
                dp = qq.tile([P, QB], f32, tag="mm", name=f"dp{qj}")
                for u in (0, 1):
                    drow = yps_t[qj][u][HD:HD + 1, :]
                    if qj == NQ - 1:
                        ld = sp.tile([1, QB], f32, tag="rec",
                                     name=f"ld{qj}{u}", bufs=6)
                        nc.scalar.activation(ld[:], drow, AF.Ln)
                        r1 = sp.tile([1, QB], bf16, tag="r1b",
                                     name=f"r1_{qj}{u}", bufs=6)
                        nc.scalar.activation(r1[:], ld[:], AF.Exp,
                                             scale=-1.0)
                        nc.tensor.matmul(dp[HD * u:HD * (u + 1), :],
                                         pm64_sb[0:1, HD:2 * HD], r1[:],
                                         start=True, stop=True)
                        continue
                    r0 = sp.tile([1, QB], f32, tag="rec", name=f"r0_{qj}{u}",
                                 bufs=6)
                    nc.vector.tensor_tensor(
                        r0[:].bitcast(i32), magic_sb[:], drow.bitcast(i32),
                        ALU.subtract)
                    t = sp.tile([1, QB], f32, tag="rec", name=f"t{qj}{u}",
                                bufs=6)
                    nc.vector.tensor_mul(t[:], drow, r0[:])
                    r1n = sp.tile([1, QB], bf16, tag="r1b",
                                  name=f"r1_{qj}{u}", bufs=6)
                    nc.vector.scalar_tensor_tensor(
                        r1n[:], t[:], 2.0, r0[:], ALU.subtract, ALU.mult)
                    nc.tensor.matmul(dp[HD * u:HD * (u + 1), :],
                                     pm64_sb[0:1, 0:HD], r1n[:],
                                     start=True, stop=True)
                dps_t[qj] = dp

            def emit_ynorms(qj):
                ysb = sp.tile([P, QB], bf16, tag="y", name=f"ysb{qj}",
                              bufs=2)
                for u in (0, 1):
                    # DVE may read only one PSUM operand per op: copy, then
                    # multiply by the broadcast reciprocal.
                    nc.vector.tensor_copy(
                        ysb[HD * u:HD * (u + 1), :], yps_t[qj][u][0:HD, :])
                nc.vector.tensor_mul(
                    ysb[:], ysb[:], dps_t.pop(qj)[:])
                yps_t.pop(qj)
                ysb_t[qj] = ysb

            outT3 = outT.rearrange("(k p) t -> p k t", p=P)

            def emit_C(qj):
                ysb = ysb_t.pop(qj)
                ot = sp.tile([P, 4 * QB], bf16, tag="ot", name=f"ot{qj}",
                             bufs=2)
                for i in range(4):
                    ops = qq.tile([P, QB], f32, tag="mm", name=f"op{qj}{i}")
                    nc.tensor.matmul(
                        ops[:], wpT_sb[:, i * P:(i + 1) * P], ysb[:],
                        start=True, stop=True)
                    nc.vector.tensor_copy(
                        ot[:, i * QB:(i + 1) * QB], ops[:])
                nc.sync.dma_start(
                    out=outT3[:, :, qj * QB:(qj + 1) * QB],
                    in_=ot[:].rearrange("p (k t) -> p k t", t=QB))

            # ---- software-pipelined schedule ----
            emit_A(0)
            pending = None
            start_idx = 0
            for qj in range(NQ):
                nki = 4 * qj + 4
                for idx in range(start_idx, nki):
                    if qj < 3 and idx == nki - 1:
                        emit_A(qj + 1)
                    emit_QKexp(qj, idx)
                    if pending is not None:
                        emit_AV(*pending)
                    pending = (qj, idx)
                if qj < 3:
                    emit_QKexp(qj + 1, 0)
                    emit_AV(*pending)            # (qj, last)
                    emit_recips_dps(qj)
                    emit_QKexp(qj + 1, 1)
                    emit_ynorms(qj)
                    emit_AV(qj + 1, 0)
                    emit_C(qj)
                    pending = (qj + 1, 1)
                    start_idx = 2
                else:
                    emit_AV(*pending)
                    emit_recips_dps(qj)
                    emit_ynorms(qj)
                    emit_C(qj)

    import concourse.mybir as mybir2
    _split_multi_waits(nc, mybir2)
    return nc


def _get_nc(with_bias=False):
    key = f"nc{int(with_bias)}"
    if key not in _cache:
        _cache[key] = _build_nc(with_bias)
    return _cache[key]


def _make_in_maps(x, weight, Wv, bv, Wp, bp, state):
    x = np.asarray(x, np.float32)
    w = np.asarray(weight, np.float32)[:, :, 0]
    if not int(np.asarray(state)):
        w = np.ones_like(w)
    WvT = np.asarray(Wv, np.float32).T
    WpT = np.asarray(Wp, np.float32).T
    bv = np.asarray(bv, np.float32)
    scale = 1.0 / np.sqrt(HD)

    in_maps = []
    for core in range(8):
        b, hp = core // 4, core % 4
        js = slice(P * hp, P * (hp + 1))
        xTb = x[b].T.reshape(4, P, T).transpose(1, 0, 2).reshape(P, 4 * T)
        wvpb = WvT[:, js].reshape(4, P, P).transpose(1, 0, 2).reshape(P, C)
        smf = np.empty((P, 1 + 2 * NKB), np.float32)
        smf[:, 0] = bv[js]
        smf[:, 1:1 + NKB] = (w[b] * scale).reshape(NKB, P).T
        smf[:, 1 + NKB:] = w[b].reshape(NKB, P).T
        in_maps.append({
            "xTp": np.ascontiguousarray(xTb).astype(ml_dtypes.bfloat16),
            "wvp": np.ascontiguousarray(wvpb).astype(ml_dtypes.bfloat16),
            "wpT": np.ascontiguousarray(WpT[js, :]).astype(
                ml_dtypes.bfloat16),
            "smf": smf,
        })
    return in_maps


def _gather(results, x=None, bp=None):
    out = np.empty((B, T, C), np.float32)
    for b in range(B):
        acc = np.zeros((C, T), np.float32)
        for hp in range(4):
            acc += results[4 * b + hp]["outT"].astype(np.float32)
        out[b] = acc.T
    if bp is not None:
        out += np.asarray(bp, np.float32)[None, None, :]
    return out


def _run(in_maps, with_bias=False, **kw):
    from concourse.bass_utils import run_bass_kernel_spmd
    return run_bass_kernel_spmd(
        _get_nc(with_bias), in_maps, list(range(8)), **kw)


def kernel(x, weight, Wv, bv, Wp, bp, state):
    in_maps = _make_in_maps(x, weight, Wv, bv, Wp, bp, state)
    res = _run(in_maps, with_bias=bool(np.any(np.asarray(bv))))
    return _gather(res.results, x, bp)

